# revision 20
# baseline (speedup 1.0000x reference)
"""BertSelfAttention (B=4, S=2048, H=1024, NH=16, HD=64) on 8 Trainium2 NeuronCores.

Sharding: batch (4) x head-group (2) -> 8 cores. Core c handles batch b=c//2 and
heads [g*8, g*8+8) with g=c%2 (output channels [g*512, (g+1)*512)).

Per-core math (all on device):
  QT[ch, s] = (wq_c @ x_b^T + bq_c)/64,  KT = wk_c @ x_b^T + bk_c
  V[s, ch]  = (x_b @ wv_c^T + bv_c)          (tokens on partitions)
  per (head h, query half ih), per key tile st (128 keys j):
      scoresT[j, i] = score/64 -> [128, 1024] PSUM (2 matmuls), 3-tag rotation
      e = C * exp(score/8):  11/16 tiles on the ACT engine
          (exp(8*in + ln C)), 5/16 on the Vector engine via a custom
          two-instruction polynomial ((v^2+av+b)(v^2+gv+d))^8 = C*e^{8v}.
          The common factor C cancels in the softmax normalization.
      ctxT[d, i] += [v_h | 1]^T-weighted e    (fused denominator row)
  ctx PSUM is single-buffered; its drain runs on the (otherwise idle) GpSimd
  engine. Device emits unnormalized ctxT + denom rows [8*65, 2048]; the host
  divides and transposes into [B, S, H].

Matmuls run in bf16 (inputs cast on the host; gate is 2e-2, this lands ~1e-2).
A nonzero attention mask is folded into V as e^mask row scaling (exact), so
the exp path never sees it and the fast zero-mask build skips it entirely.
"""

import os
import sys

if "/opt/trn_rl_repo" not in sys.path:
    sys.path.insert(0, "/opt/trn_rl_repo")

import numpy as np
import ml_dtypes

BF16 = ml_dtypes.bfloat16

_KERNEL_DIR = os.path.dirname(os.path.abspath(__file__))

B, S, H = 4, 2048, 1024
NH, HD = 16, 64
HPC = 8          # heads per core
CH = HPC * HD    # 512 output channels per core
CT = H // 128    # 8 contraction tiles
JT = CH // 128   # 4 channel tiles per core
ST = S // 128    # 16 token tiles
VW = HD + 1      # 65: v columns + fused ones column

# exp approximation: ((v^2+A v+B)(v^2+G v+D))^8 ~ K8 * e^{8v}, |8v| <= 6.8
EXP8_A = 3.679744107637691
EXP8_B = 4.1035325847517505
EXP8_G = 0.6152512835040944
EXP8_D = 6.017592902685957
LN_K8 = 25.653546401996574

# Per-block exp engine split (16 key tiles): GpSimd-assisted tiles are
# computed first and consumed last (PSUM accumulation is order-independent),
# hiding the slow 3-pass squaring. ACT takes the rest.
GP_ST = (0, 1)           # exp8q on DVE, the ^8 squarings on GpSimd
DVE_ST = (3, 6, 9)       # exp fully on the Vector engine
# ctx consumption schedule: emission slot k (after scores/exp of tile k)
# -> which tile's ctx matmuls to emit; remaining tiles drain after the loop.
CTX_SLOT = {4: 2, 6: 4, 7: 3, 8: 5, 9: 7, 10: 6, 11: 8, 12: 10, 13: 9,
            14: 11, 15: 12}
CTX_POST = (13, 14, 15, 0, 1)

_CACHE = {}


def _register_dve_ops():
    """Register the EXP8Q/POW8 custom Vector-engine ops (idempotent)."""
    from concourse import dve_ops
    from concourse.dve_spec import (Spec, Src0, C0, C1, C2, C3, sq, lower,
                                    _spill_c3_to_src1)
    from concourse.dve_uop import DveOpSpec

    def _ref_exp8q(in0, in1, s0, s1, imm2):
        v = in0.astype(np.float32)
        f2 = v * v
        return ((f2 + s0 * v + s1) * (f2 + imm2 * v + in1)).astype(np.float32)

    def _ref_pow8(in0, in1, s0, s1, imm2):
        p = in0.astype(np.float32)
        p = p * p
        p = p * p
        return (p * p).astype(np.float32)

    v = Src0
    f2 = sq(v)
    defs = [
        ("EXP8Q_ANT",
         _spill_c3_to_src1((f2 + (v * C0 + C1)) * (f2 + (v * C2 + C3))),
         _ref_exp8q, True),
        ("POW8_ANT", sq(sq(sq(Src0))), _ref_pow8, False),
    ]
    out = []
    for name, body, ref, rd1 in defs:
        existing = [op for op in dve_ops.OPS if op.name == name]
        if existing:
            out.append(existing[0])
            continue
        spec = Spec(body=body, reference=ref)
        row = dve_ops._CUSTOM_DVE_ROW_BASE + len(dve_ops.OPS)
        assert row < 0x20
        dve_ops._SUB_OPCODE_FOR_NAME[name] = row
        shas = {}
        for ver in ("v3", "v4"):
            s = DveOpSpec(name=name, opcode=row, uops=lower(spec, ver=ver),
                          rd1_en=rd1)
            shas[ver] = s.sha(ver)
        op = dve_ops.DveOp(name, spec, subdim=False, uops_sha=shas)
        dve_ops.OPS.append(op)
        dve_ops.CUSTOM_DVE_SPECS[name] = spec
        out.append(op)
    return out


def _build(apply_mask=False):
    import concourse.bass as bass  # noqa: F401  (registers engine methods)
    import concourse.mybir as mybir
    import concourse.tile as tile
    from concourse import bacc

    F32 = mybir.dt.float32
    BF = mybir.dt.bfloat16
    exp8q, pow8 = _register_dve_ops()

    nc = bacc.Bacc("TRN2", target_bir_lowering=False, debug=True)

    xt = nc.dram_tensor("xt", [H, S], BF, kind="ExternalInput")        # x_b^T
    wq_t = nc.dram_tensor("wq_t", [H, CH], BF, kind="ExternalInput")   # wq_c^T
    wk_t = nc.dram_tensor("wk_t", [H, CH], BF, kind="ExternalInput")
    wv_t = nc.dram_tensor("wv_t", [H, CH], BF, kind="ExternalInput")
    bq = nc.dram_tensor("bq", [CH], F32, kind="ExternalInput")   # pre-scaled /64
    bk = nc.dram_tensor("bk", [CH], F32, kind="ExternalInput")
    bv = nc.dram_tensor("bv", [CH], F32, kind="ExternalInput")
    mask = nc.dram_tensor("mask", [S], F32, kind="ExternalInput")
    ones = nc.dram_tensor("ones", [512], BF, kind="ExternalInput")
    # unnormalized ctxT + denominator rows, 65 rows per head
    out = nc.dram_tensor("out", [VW * HPC, S], F32, kind="ExternalOutput")

    with tile.TileContext(nc) as tc, nc.allow_low_precision(reason="bf16 attention"):
        from contextlib import ExitStack

        with ExitStack() as outer:
            persist = outer.enter_context(tc.tile_pool(name="persist", bufs=1))
            ppool = outer.enter_context(tc.tile_pool(name="pp", bufs=1, space="PSUM"))

            # Persistent SBUF tensors
            # Q per head, zero-padded to 128 partitions (head h lives in its own
            # partition range po:po+64; the other 64 rows are zeros).
            qp_sb = [persist.tile([128, S], BF, tag=f"qp{h}", name=f"qp{h}")
                     for h in range(HPC)]
            kt_sb = [persist.tile([128, S], BF, tag=f"kt{j}", name=f"kt{j}")
                     for j in range(JT)]
            v_sb = persist.tile([128, ST, VW * HPC], BF, tag="v")
            bqp = persist.tile([128, JT], F32, tag="bqp")
            bkp = persist.tile([128, JT], F32, tag="bkp")
            bv_bc = persist.tile([128, CH], F32, tag="bv_bc")
            ones8 = persist.tile([128, HPC], BF, tag="ones8")
            zcol = persist.tile([128, 1], BF, tag="zcol")
            lnk8 = persist.tile([128, 1], F32, tag="lnk8")
            dcol = persist.tile([128, 1], F32, tag="dcol")
            if apply_mask:
                mask_sb = persist.tile([128, ST], F32, tag="mask")
                em_sb = persist.tile([128, ST], F32, tag="em")

            def _misc_dmas():
                """Issued after the critical-path x/w DMAs: nothing here is
                needed before the first PSUM drain (~20us in)."""
                if apply_mask:
                    nc.sync.dma_start(out=mask_sb,
                                      in_=mask.rearrange("(t p) -> p t", p=128))
                    nc.scalar.activation(em_sb, mask_sb,
                                         mybir.ActivationFunctionType.Exp)
                nc.sync.dma_start(out=bqp,
                                  in_=bq.rearrange("(j p) -> p j", p=128))
                nc.sync.dma_start(out=bkp,
                                  in_=bk.rearrange("(j p) -> p j", p=128))
                nc.sync.dma_start(
                    out=bv_bc,
                    in_=bass.AP(tensor=bv, offset=0, ap=[[0, 128], [1, CH]]))
                nc.sync.dma_start(
                    out=ones8,
                    in_=bass.AP(tensor=ones, offset=0, ap=[[0, 128], [1, HPC]]))
                # ones columns of v (position 64 of each head block, every
                # token tile); under apply_mask they carry e^mask instead.
                for t in range(ST):
                    if apply_mask:
                        em_b = bass.AP(tensor=em_sb.tensor,
                                       offset=em_sb[:, t:t + 1].offset,
                                       ap=[em_sb.ap[0], [0, HPC]])
                        nc.vector.tensor_copy(v4[:, t, :, HD], em_b)
                    else:
                        nc.vector.tensor_copy(v4[:, t, :, HD], ones8)
                # zero the unused partition half of each padded-Q tile
                nc.vector.memset(zcol, 0.0)
                for hh in range(HPC):
                    zo = 64 if hh % 2 == 0 else 0  # rows NOT owned by head hh
                    zsrc = zcol[zo:zo + 64, 0:1]
                    zbcast = bass.AP(tensor=zsrc.tensor, offset=zsrc.offset,
                                     ap=[zsrc.ap[0], [0, S]])
                    nc.vector.tensor_copy(qp_sb[hh][zo:zo + 64, :], zbcast)

            nc.vector.memset(lnk8, LN_K8)
            nc.vector.memset(dcol, EXP8_D)
            v4 = v_sb.rearrange("p t (h e) -> p t h e", e=VW)

            # ---------------- Phase 1: QKV projections ----------------
            with ExitStack() as ph1:
                wpool = ph1.enter_context(tc.tile_pool(name="w", bufs=1))

                # x kept fully SBUF-resident: 4 quarter DMAs [128, CT, 512];
                # the V pass reuses it instead of re-streaming from HBM.
                x_sb = wpool.tile([128, CT, S], BF, tag="xall", name="xall")
                wq_sb = wpool.tile([128, CT, CH], BF, tag="wqall", name="wqall")
                wk_sb = wpool.tile([128, CT, CH], BF, tag="wkall", name="wkall")
                wv_sb = wpool.tile([128, CT, CH], BF, tag="wvall", name="wvall")

                x_r = xt.rearrange("(c p) (q n) -> p c q n", p=128, n=512)
                wq_r = wq_t.rearrange("(c p) j -> p c j", p=128)
                wk_r = wk_t.rearrange("(c p) j -> p c j", p=128)
                wv_r = wv_t.rearrange("(c p) j -> p c j", p=128)
                x4 = x_sb.rearrange("p c (q n) -> p c q n", n=512)

                # critical-path first: the first (j,sq)-block consumes x
                # quarter 0 and wq/wk progressively by ct — split the loads
                # so early matmuls start while the bulk still streams.
                nc.sync.dma_start(out=x4[:, 0, 0, :], in_=x_r[:, 0, 0, :])
                nc.sync.dma_start(out=wq_sb[:, 0, :], in_=wq_r[:, 0, :])
                nc.sync.dma_start(out=wk_sb[:, 0, :], in_=wk_r[:, 0, :])
                nc.sync.dma_start(out=x4[:, 1:4, 0, :], in_=x_r[:, 1:4, 0, :])
                nc.sync.dma_start(out=wq_sb[:, 1:4, :], in_=wq_r[:, 1:4, :])
                nc.sync.dma_start(out=wk_sb[:, 1:4, :], in_=wk_r[:, 1:4, :])
                nc.sync.dma_start(out=x4[:, 4:CT, 0, :], in_=x_r[:, 4:CT, 0, :])
                nc.sync.dma_start(out=wq_sb[:, 4:CT, :], in_=wq_r[:, 4:CT, :])
                nc.sync.dma_start(out=wk_sb[:, 4:CT, :], in_=wk_r[:, 4:CT, :])
                _misc_dmas()
                for qi_ in range(1, 4):
                    nc.sync.dma_start(out=x4[:, :, qi_, :], in_=x_r[:, :, qi_, :])
                nc.sync.dma_start(out=wv_sb[:, :, :], in_=wv_r[:, :, :])

                # Combined Q+K pass as 16 (j, sq) blocks on a 4-deep PSUM tag
                # rotation (tag == j): drains of block b have ~3 blocks of
                # slack before tag reuse — no quarter-boundary barriers.
                # PSUM tag t{j} holds Q_j in columns 0:512 and K_j in 512:1024.
                for b in range(16):
                    j, sq_i = b % 4, b // 4
                    pqk = ppool.tile([128, 1024], F32, tag=f"t{j}",
                                     name=f"pqk{sq_i}{j}")
                    x_t = x4[:, :, sq_i, :]
                    for ct in range(CT):
                        nc.tensor.matmul(
                            pqk[:, 0:512],
                            lhsT=wq_sb[:, ct, j * 128:(j + 1) * 128],
                            rhs=x_t[:, ct, :],
                            start=(ct == 0), stop=(ct == CT - 1))
                        nc.tensor.matmul(
                            pqk[:, 512:1024],
                            lhsT=wk_sb[:, ct, j * 128:(j + 1) * 128],
                            rhs=x_t[:, ct, :],
                            start=(ct == 0), stop=(ct == CT - 1))
                    # drain across ACT and DVE (balanced): q head-even +
                    # k-even on ACT, q head-odd + k-odd on DVE. q is
                    # pre-scaled by 1/64 (bq arrives pre-scaled).
                    h0, h1 = 2 * j, 2 * j + 1
                    nc.scalar.activation(
                        qp_sb[h0][0:64, sq_i * 512:(sq_i + 1) * 512],
                        pqk[0:64, 0:512],
                        mybir.ActivationFunctionType.Identity,
                        bias=bqp[0:64, j:j + 1], scale=1.0 / 64)
                    nc.vector.tensor_scalar(
                        qp_sb[h1][64:128, sq_i * 512:(sq_i + 1) * 512],
                        pqk[64:128, 0:512],
                        1.0 / 64, bqp[64:128, j:j + 1],
                        mybir.AluOpType.mult, mybir.AluOpType.add)
                    if j % 2 == 0:
                        nc.scalar.activation(
                            kt_sb[j][:, sq_i * 512:(sq_i + 1) * 512],
                            pqk[:, 512:1024],
                            mybir.ActivationFunctionType.Identity,
                            bias=bkp[:, j:j + 1], scale=1.0)
                    else:
                        nc.vector.tensor_scalar_add(
                            kt_sb[j][:, sq_i * 512:(sq_i + 1) * 512],
                            pqk[:, 512:1024],
                            bkp[:, j:j + 1])

                # V pass: tokens on psum partitions (x already resident),
                # 8 (sh, jj) blocks continuing the 4-deep tag rotation.
                for b in range(8):
                    sh, jj = b // 4, b % 4
                    pv = ppool.tile([128, 1024], F32, tag=f"t{jj}",
                                    name=f"pv{sh}{jj}")
                    for ct in range(CT):
                        for half in range(2):
                            s0 = sh * 1024 + (2 * jj + half) * 128
                            nc.tensor.matmul(
                                pv[:, half * 512:(half + 1) * 512],
                                lhsT=x_sb[:, ct, s0:s0 + 128],
                                rhs=wv_sb[:, ct, :],
                                start=(ct == 0), stop=(ct == CT - 1))
                    for half in range(2):
                        t_idx = sh * 8 + 2 * jj + half
                        sl = pv[:, half * 512:(half + 1) * 512]
                        # one 8-head add per token tile via 3D APs
                        dst = v4[:, t_idx, :, 0:HD]
                        sl3 = bass.AP(tensor=sl.tensor, offset=sl.offset,
                                      ap=[sl.ap[0], [HD, HPC], [1, HD]])
                        bv3 = bass.AP(tensor=bv_bc.tensor, offset=bv_bc.offset,
                                      ap=[bv_bc.ap[0], [HD, HPC], [1, HD]])
                        nc.vector.tensor_add(dst, sl3, bv3)
                        if apply_mask:
                            nc.vector.tensor_scalar_mul(
                                dst, dst, em_sb[:, t_idx:t_idx + 1])

            # ---------------- Phase 2: attention ----------------
            with ExitStack() as ph2:
                epool = ph2.enter_context(tc.tile_pool(name="ep", bufs=8))
                e01pool = ph2.enter_context(tc.tile_pool(name="e01", bufs=4))
                tpool = ph2.enter_context(tc.tile_pool(name="tp", bufs=3))
                gtpool = ph2.enter_context(tc.tile_pool(name="gt", bufs=3))
                gpool = ph2.enter_context(tc.tile_pool(name="gp", bufs=4))
                opool = ph2.enter_context(tc.tile_pool(name="op", bufs=3))

                for h in range(HPC):
                    qi = h // 2
                    for ih in range(2):
                        blk = h * 2 + ih
                        i0 = ih * 1024
                        ctx_ps = ppool.tile([VW, 1024], F32, tag="t3",
                                            name=f"ctx{blk}")
                        e_tiles = {}
                        emitted = []

                        def _ctx(st):
                            e_t = e_tiles.pop(st)
                            first = not emitted
                            emitted.append(st)
                            last = len(emitted) == ST
                            for q in range(2):
                                nc.tensor.matmul(
                                    ctx_ps[:, q * 512:(q + 1) * 512],
                                    lhsT=v_sb[:, st, h * VW:(h + 1) * VW],
                                    rhs=e_t[:, q * 512:(q + 1) * 512],
                                    start=first, stop=last)

                        for st in range(ST):
                            # per-block tag offset: the first tiles of block
                            # N+1 reuse tags whose exp finished early in block
                            # N — no boundary stall.
                            s_ps = ppool.tile([128, 1024], F32,
                                              tag=f"t{(st + blk) % 3}",
                                              name=f"sc{blk}_{st}")
                            for q in range(2):
                                nc.tensor.matmul(
                                    s_ps[:, q * 512:(q + 1) * 512],
                                    lhsT=kt_sb[qi][:, st * 128:(st + 1) * 128],
                                    rhs=qp_sb[h][:, i0 + q * 512:i0 + (q + 1) * 512],
                                    start=True, stop=True)
                            if st in GP_ST:
                                # exp8q on DVE, the three ^2 passes on GpSimd
                                e_sb = e01pool.tile([128, 1024], BF, tag="e01",
                                                    name=f"eg{blk}_{st}")
                                tmp = gtpool.tile([128, 1024], F32, tag="gtmp",
                                                  name=f"gt{blk}_{st}")
                                nc.vector._custom_dve(
                                    exp8q, out=tmp, in0=s_ps, in1=dcol,
                                    s0=EXP8_A, s1=EXP8_B, imm2=EXP8_G)
                                g1 = gpool.tile([128, 1024], F32, tag="g",
                                                name=f"g1_{blk}_{st}")
                                g2 = gpool.tile([128, 1024], F32, tag="g",
                                                name=f"g2_{blk}_{st}")
                                nc.gpsimd.tensor_mul(g1, tmp, tmp)
                                nc.gpsimd.tensor_mul(g2, g1, g1)
                                nc.gpsimd.tensor_mul(e_sb, g2, g2)
                            elif st in DVE_ST:
                                e_sb = epool.tile([128, 1024], BF, tag="e",
                                                  name=f"e{blk}_{st}")
                                tmp = tpool.tile([128, 1024], F32, tag="tmp",
                                                 name=f"tm{blk}_{st}")
                                nc.vector._custom_dve(
                                    exp8q, out=tmp, in0=s_ps, in1=dcol,
                                    s0=EXP8_A, s1=EXP8_B, imm2=EXP8_G)
                                nc.vector._custom_dve(pow8, out=e_sb, in0=tmp)
                            else:
                                e_sb = epool.tile([128, 1024], BF, tag="e",
                                                  name=f"e{blk}_{st}")
                                nc.scalar.activation(
                                    e_sb, s_ps,
                                    mybir.ActivationFunctionType.Exp,
                                    bias=lnk8, scale=8.0)
                            e_tiles[st] = e_sb
                            if st in CTX_SLOT:
                                _ctx(CTX_SLOT[st])
                        for st in CTX_POST:
                            _ctx(st)
                        # drain + store in halves so the DMA overlaps the
                        # second half's copy
                        o_sb = opool.tile([VW, 1024], F32, tag="o", name=f"o{blk}")
                        for hf in range(2):
                            sl = slice(hf * 512, (hf + 1) * 512)
                            nc.vector.tensor_copy(o_sb[:, sl], ctx_ps[:, sl])
                            nc.sync.dma_start(
                                out=out[h * VW:(h + 1) * VW,
                                        i0 + hf * 512:i0 + (hf + 1) * 512],
                                in_=o_sb[:, sl])

    nc.compile()
    return nc


def _get_nc(apply_mask=False):
    key = ("nc", apply_mask)
    if key not in _CACHE:
        _CACHE[key] = _build(apply_mask)
    return _CACHE[key]


def _in_maps(hidden_states, attention_mask, wq, bq, wk, bk, wv, bv):
    ones = np.ones(512, BF16)
    maps = []
    for c in range(8):
        b, g = c // 2, c % 2
        ch0 = g * CH
        maps.append({
            "xt": np.ascontiguousarray(hidden_states[b].T).astype(BF16),
            "wq_t": np.ascontiguousarray(wq[ch0:ch0 + CH, :].T).astype(BF16),
            "wk_t": np.ascontiguousarray(wk[ch0:ch0 + CH, :].T).astype(BF16),
            "wv_t": np.ascontiguousarray(wv[ch0:ch0 + CH, :].T).astype(BF16),
            "bq": np.ascontiguousarray(bq[ch0:ch0 + CH]) / 64.0,
            "bk": np.ascontiguousarray(bk[ch0:ch0 + CH]),
            "bv": np.ascontiguousarray(bv[ch0:ch0 + CH]),
            "mask": np.ascontiguousarray(attention_mask[b, 0, 0, :]),
            "ones": ones,
        })
    return maps


def _gather(results):
    full = np.empty((B, S, H), np.float32)
    for c in range(8):
        b, g = c // 2, c % 2
        o = results[c]["out"].reshape(HPC, VW, S)
        ctx = o[:, :HD, :] / o[:, HD:HD + 1, :]        # normalize by denom row
        # [h, d, s] -> [s, h*d]
        full[b, :, g * CH:(g + 1) * CH] = ctx.reshape(CH, S).T
    return full


def _run(in_maps, trace=False, apply_mask=False):
    from concourse.bass_utils import run_bass_kernel_spmd

    nc = _get_nc(apply_mask)
    return run_bass_kernel_spmd(nc, in_maps, list(range(8)), trace=trace)


def _needs_mask(attention_mask):
    return bool(np.any(np.asarray(attention_mask) != 0.0))


def _run_results(in_maps, apply_mask=False):
    """Run on hardware; on a wedged-device error retry in fresh subprocesses
    (the PJRT client cannot recover an unrecoverable exec unit in-process)."""
    try:
        return _run(in_maps, apply_mask=apply_mask).results
    except Exception:
        pass
    import pickle
    import subprocess
    import tempfile

    last = None
    for _ in range(3):
        try:
            with tempfile.TemporaryDirectory() as td:
                fin = os.path.join(td, "in.pkl")
                fout = os.path.join(td, "out.pkl")
                with open(fin, "wb") as f:
                    pickle.dump((in_maps, apply_mask), f)
                code = (
                    "import pickle, sys\n"
                    f"sys.path.insert(0, {_KERNEL_DIR!r})\n"
                    "import kernel\n"
                    f"maps, am = pickle.load(open({fin!r}, 'rb'))\n"
                    "res = kernel._run(maps, apply_mask=am)\n"
                    f"pickle.dump(res.results, open({fout!r}, 'wb'))\n"
                )
                subprocess.run([sys.executable, "-c", code], check=True,
                               timeout=1800)
                with open(fout, "rb") as f:
                    return pickle.load(f)
        except Exception as e:
            last = e
    raise last


def kernel(hidden_states, attention_mask, wq, bq, wk, bk, wv, bv):
    args = [np.asarray(a, np.float32) for a in
            (hidden_states, attention_mask, wq, bq, wk, bk, wv, bv)]
    am = _needs_mask(args[1])
    return _gather(_run_results(_in_maps(*args), apply_mask=am))


def kernel_profiled(hidden_states, attention_mask, wq, bq, wk, bk, wv, bv):
    """Like kernel() but with NTFF tracing; returns (output, exec_time_ns)."""
    args = [np.asarray(a, np.float32) for a in
            (hidden_states, attention_mask, wq, bq, wk, bk, wv, bv)]
    am = _needs_mask(args[1])
    res = _run(_in_maps(*args), trace=True, apply_mask=am)
    return _gather(res.results), res.exec_time_ns


# revision 21
# speedup vs baseline: 1.0091x; 1.0091x over previous
"""BertSelfAttention (B=4, S=2048, H=1024, NH=16, HD=64) on 8 Trainium2 NeuronCores.

Sharding: batch (4) x head-group (2) -> 8 cores. Core c handles batch b=c//2 and
heads [g*8, g*8+8) with g=c%2 (output channels [g*512, (g+1)*512)).

Per-core math (all on device):
  QT[ch, s] = (wq_c @ x_b^T + bq_c)/64,  KT = wk_c @ x_b^T + bk_c
  V[s, ch]  = (x_b @ wv_c^T + bv_c)          (tokens on partitions)
  per (head h, query half ih), per key tile st (128 keys j):
      scoresT[j, i] = score/64 -> [128, 1024] PSUM (2 matmuls), 3-tag rotation
      e = C * exp(score/8):  11/16 tiles on the ACT engine
          (exp(8*in + ln C)), 5/16 on the Vector engine via a custom
          two-instruction polynomial ((v^2+av+b)(v^2+gv+d))^8 = C*e^{8v}.
          The common factor C cancels in the softmax normalization.
      ctxT[d, i] += [v_h | 1]^T-weighted e    (fused denominator row)
  ctx PSUM is single-buffered; its drain runs on the (otherwise idle) GpSimd
  engine. Device emits unnormalized ctxT + denom rows [8*65, 2048]; the host
  divides and transposes into [B, S, H].

Matmuls run in bf16 (inputs cast on the host; gate is 2e-2, this lands ~1e-2).
A nonzero attention mask is folded into V as e^mask row scaling (exact), so
the exp path never sees it and the fast zero-mask build skips it entirely.
"""

import os
import sys

if "/opt/trn_rl_repo" not in sys.path:
    sys.path.insert(0, "/opt/trn_rl_repo")

import numpy as np
import ml_dtypes

BF16 = ml_dtypes.bfloat16

_KERNEL_DIR = os.path.dirname(os.path.abspath(__file__))

B, S, H = 4, 2048, 1024
NH, HD = 16, 64
HPC = 8          # heads per core
CH = HPC * HD    # 512 output channels per core
CT = H // 128    # 8 contraction tiles
JT = CH // 128   # 4 channel tiles per core
ST = S // 128    # 16 token tiles
VW = HD + 1      # 65: v columns + fused ones column

# exp approximation: ((v^2+A v+B)(v^2+G v+D))^8 ~ K8 * e^{8v}, |8v| <= 6.8
EXP8_A = 3.679744107637691
EXP8_B = 4.1035325847517505
EXP8_G = 0.6152512835040944
EXP8_D = 6.017592902685957
LN_K8 = 25.653546401996574

# Per-block exp engine split (16 key tiles): GpSimd-assisted tiles are
# computed first and consumed last (PSUM accumulation is order-independent),
# hiding the slow 3-pass squaring. ACT takes the rest.
GP_ST = (0, 1)           # exp8q on DVE, the ^8 squarings on GpSimd
DVE_ST = (3, 6, 9)       # exp fully on the Vector engine
# ctx consumption schedule: emission slot k (after scores/exp of tile k)
# -> which tile's ctx matmuls to emit; remaining tiles drain after the loop.
CTX_SLOT = {4: 2, 6: 4, 7: 3, 8: 5, 9: 7, 10: 6, 11: 8, 12: 10, 13: 9,
            14: 11, 15: 12}
CTX_POST = (13, 14, 15, 0, 1)

_CACHE = {}


def _register_dve_ops():
    """Register the EXP8Q/POW8 custom Vector-engine ops (idempotent)."""
    from concourse import dve_ops
    from concourse.dve_spec import (Spec, Src0, C0, C1, C2, C3, sq, lower,
                                    _spill_c3_to_src1)
    from concourse.dve_uop import DveOpSpec

    def _ref_exp8q(in0, in1, s0, s1, imm2):
        v = in0.astype(np.float32)
        f2 = v * v
        return ((f2 + s0 * v + s1) * (f2 + imm2 * v + in1)).astype(np.float32)

    def _ref_pow8(in0, in1, s0, s1, imm2):
        p = in0.astype(np.float32)
        p = p * p
        p = p * p
        return (p * p).astype(np.float32)

    v = Src0
    f2 = sq(v)
    defs = [
        ("EXP8Q_ANT",
         _spill_c3_to_src1((f2 + (v * C0 + C1)) * (f2 + (v * C2 + C3))),
         _ref_exp8q, True),
        ("POW8_ANT", sq(sq(sq(Src0))), _ref_pow8, False),
    ]
    out = []
    for name, body, ref, rd1 in defs:
        existing = [op for op in dve_ops.OPS if op.name == name]
        if existing:
            out.append(existing[0])
            continue
        spec = Spec(body=body, reference=ref)
        row = dve_ops._CUSTOM_DVE_ROW_BASE + len(dve_ops.OPS)
        assert row < 0x20
        dve_ops._SUB_OPCODE_FOR_NAME[name] = row
        shas = {}
        for ver in ("v3", "v4"):
            s = DveOpSpec(name=name, opcode=row, uops=lower(spec, ver=ver),
                          rd1_en=rd1)
            shas[ver] = s.sha(ver)
        op = dve_ops.DveOp(name, spec, subdim=False, uops_sha=shas)
        dve_ops.OPS.append(op)
        dve_ops.CUSTOM_DVE_SPECS[name] = spec
        out.append(op)
    return out


def _build(apply_mask=False):
    import concourse.bass as bass  # noqa: F401  (registers engine methods)
    import concourse.mybir as mybir
    import concourse.tile as tile
    from concourse import bacc

    F32 = mybir.dt.float32
    BF = mybir.dt.bfloat16
    exp8q, pow8 = _register_dve_ops()

    nc = bacc.Bacc("TRN2", target_bir_lowering=False, debug=True)

    xt = nc.dram_tensor("xt", [H, S], BF, kind="ExternalInput")        # x_b^T
    wq_t = nc.dram_tensor("wq_t", [H, CH], BF, kind="ExternalInput")   # wq_c^T
    wk_t = nc.dram_tensor("wk_t", [H, CH], BF, kind="ExternalInput")
    wv_t = nc.dram_tensor("wv_t", [H, CH], BF, kind="ExternalInput")
    bq = nc.dram_tensor("bq", [CH], F32, kind="ExternalInput")   # pre-scaled /64
    bk = nc.dram_tensor("bk", [CH], F32, kind="ExternalInput")
    bv = nc.dram_tensor("bv", [CH], F32, kind="ExternalInput")
    mask = nc.dram_tensor("mask", [S], F32, kind="ExternalInput")
    ones = nc.dram_tensor("ones", [512], BF, kind="ExternalInput")
    # unnormalized ctxT + denominator rows, 65 rows per head
    out = nc.dram_tensor("out", [VW * HPC, S], F32, kind="ExternalOutput")

    with tile.TileContext(nc) as tc, nc.allow_low_precision(reason="bf16 attention"):
        from contextlib import ExitStack

        with ExitStack() as outer:
            persist = outer.enter_context(tc.tile_pool(name="persist", bufs=1))
            ppool = outer.enter_context(tc.tile_pool(name="pp", bufs=1, space="PSUM"))

            # Persistent SBUF tensors
            # Q per head, zero-padded to 128 partitions (head h lives in its own
            # partition range po:po+64; the other 64 rows are zeros).
            qp_sb = [persist.tile([128, S], BF, tag=f"qp{h}", name=f"qp{h}")
                     for h in range(HPC)]
            kt_sb = [persist.tile([128, S], BF, tag=f"kt{j}", name=f"kt{j}")
                     for j in range(JT)]
            v_sb = persist.tile([128, ST, VW * HPC], BF, tag="v")
            bqp = persist.tile([128, JT], F32, tag="bqp")
            bkp = persist.tile([128, JT], F32, tag="bkp")
            bv_bc = persist.tile([128, CH], F32, tag="bv_bc")
            ones8 = persist.tile([128, HPC], BF, tag="ones8")
            zcol = persist.tile([128, 1], BF, tag="zcol")
            lnk8 = persist.tile([128, 1], F32, tag="lnk8")
            dcol = persist.tile([128, 1], F32, tag="dcol")
            if apply_mask:
                mask_sb = persist.tile([128, ST], F32, tag="mask")
                em_sb = persist.tile([128, ST], F32, tag="em")

            def _misc_dmas():
                """Issued after the critical-path x/w DMAs: nothing here is
                needed before the first PSUM drain (~20us in)."""
                if apply_mask:
                    nc.sync.dma_start(out=mask_sb,
                                      in_=mask.rearrange("(t p) -> p t", p=128))
                    nc.scalar.activation(em_sb, mask_sb,
                                         mybir.ActivationFunctionType.Exp)
                nc.sync.dma_start(out=bqp,
                                  in_=bq.rearrange("(j p) -> p j", p=128))
                nc.sync.dma_start(out=bkp,
                                  in_=bk.rearrange("(j p) -> p j", p=128))
                nc.sync.dma_start(
                    out=bv_bc,
                    in_=bass.AP(tensor=bv, offset=0, ap=[[0, 128], [1, CH]]))
                nc.sync.dma_start(
                    out=ones8,
                    in_=bass.AP(tensor=ones, offset=0, ap=[[0, 128], [1, HPC]]))
                # ones columns of v (position 64 of each head block, every
                # token tile); under apply_mask they carry e^mask instead.
                for t in range(ST):
                    if apply_mask:
                        em_b = bass.AP(tensor=em_sb.tensor,
                                       offset=em_sb[:, t:t + 1].offset,
                                       ap=[em_sb.ap[0], [0, HPC]])
                        nc.vector.tensor_copy(v4[:, t, :, HD], em_b)
                    else:
                        nc.vector.tensor_copy(v4[:, t, :, HD], ones8)
                # zero the unused partition half of each padded-Q tile
                nc.vector.memset(zcol, 0.0)
                for hh in range(HPC):
                    zo = 64 if hh % 2 == 0 else 0  # rows NOT owned by head hh
                    zsrc = zcol[zo:zo + 64, 0:1]
                    zbcast = bass.AP(tensor=zsrc.tensor, offset=zsrc.offset,
                                     ap=[zsrc.ap[0], [0, S]])
                    nc.vector.tensor_copy(qp_sb[hh][zo:zo + 64, :], zbcast)

            nc.vector.memset(lnk8, LN_K8)
            nc.vector.memset(dcol, EXP8_D)
            v4 = v_sb.rearrange("p t (h e) -> p t h e", e=VW)

            # ---------------- Phase 1: QKV projections ----------------
            with ExitStack() as ph1:
                wpool = ph1.enter_context(tc.tile_pool(name="w", bufs=1))

                # x kept fully SBUF-resident: 4 quarter DMAs [128, CT, 512];
                # the V pass reuses it instead of re-streaming from HBM.
                x_sb = wpool.tile([128, CT, S], BF, tag="xall", name="xall")
                wq_sb = wpool.tile([128, CT, CH], BF, tag="wqall", name="wqall")
                wk_sb = wpool.tile([128, CT, CH], BF, tag="wkall", name="wkall")
                wv_sb = wpool.tile([128, CT, CH], BF, tag="wvall", name="wvall")

                x_r = xt.rearrange("(c p) (q n) -> p c q n", p=128, n=512)
                wq_r = wq_t.rearrange("(c p) j -> p c j", p=128)
                wk_r = wk_t.rearrange("(c p) j -> p c j", p=128)
                wv_r = wv_t.rearrange("(c p) j -> p c j", p=128)
                x4 = x_sb.rearrange("p c (q n) -> p c q n", n=512)

                # critical-path first: the first (j,sq)-block consumes x
                # quarter 0 and wq/wk progressively by ct — split the loads
                # so early matmuls start while the bulk still streams.
                nc.sync.dma_start(out=x4[:, 0, 0, :], in_=x_r[:, 0, 0, :])
                nc.sync.dma_start(out=wq_sb[:, 0, :], in_=wq_r[:, 0, :])
                nc.sync.dma_start(out=wk_sb[:, 0, :], in_=wk_r[:, 0, :])
                nc.sync.dma_start(out=x4[:, 1:4, 0, :], in_=x_r[:, 1:4, 0, :])
                nc.sync.dma_start(out=wq_sb[:, 1:4, :], in_=wq_r[:, 1:4, :])
                nc.sync.dma_start(out=wk_sb[:, 1:4, :], in_=wk_r[:, 1:4, :])
                nc.sync.dma_start(out=x4[:, 4:CT, 0, :], in_=x_r[:, 4:CT, 0, :])
                nc.sync.dma_start(out=wq_sb[:, 4:CT, :], in_=wq_r[:, 4:CT, :])
                nc.sync.dma_start(out=wk_sb[:, 4:CT, :], in_=wk_r[:, 4:CT, :])
                _misc_dmas()
                for qi_ in range(1, 4):
                    nc.sync.dma_start(out=x4[:, :, qi_, :], in_=x_r[:, :, qi_, :])
                nc.sync.dma_start(out=wv_sb[:, :, :], in_=wv_r[:, :, :])

                # Combined Q+K pass as 16 (j, sq) blocks on a 4-deep PSUM tag
                # rotation (tag == j): drains of block b have ~3 blocks of
                # slack before tag reuse — no quarter-boundary barriers.
                # PSUM tag t{j} holds Q_j in columns 0:512 and K_j in 512:1024.
                for b in range(16):
                    j, sq_i = b % 4, b // 4
                    pqk = ppool.tile([128, 1024], F32, tag=f"t{j}",
                                     name=f"pqk{sq_i}{j}")
                    x_t = x4[:, :, sq_i, :]
                    for ct in range(CT):
                        nc.tensor.matmul(
                            pqk[:, 0:512],
                            lhsT=wq_sb[:, ct, j * 128:(j + 1) * 128],
                            rhs=x_t[:, ct, :],
                            start=(ct == 0), stop=(ct == CT - 1))
                        nc.tensor.matmul(
                            pqk[:, 512:1024],
                            lhsT=wk_sb[:, ct, j * 128:(j + 1) * 128],
                            rhs=x_t[:, ct, :],
                            start=(ct == 0), stop=(ct == CT - 1))
                    # drain across ACT and DVE (balanced): q head-even +
                    # k-even on ACT, q head-odd + k-odd on DVE. q is
                    # pre-scaled by 1/64 (bq arrives pre-scaled).
                    h0, h1 = 2 * j, 2 * j + 1
                    nc.scalar.activation(
                        qp_sb[h0][0:64, sq_i * 512:(sq_i + 1) * 512],
                        pqk[0:64, 0:512],
                        mybir.ActivationFunctionType.Identity,
                        bias=bqp[0:64, j:j + 1], scale=1.0 / 64)
                    nc.vector.tensor_scalar(
                        qp_sb[h1][64:128, sq_i * 512:(sq_i + 1) * 512],
                        pqk[64:128, 0:512],
                        1.0 / 64, bqp[64:128, j:j + 1],
                        mybir.AluOpType.mult, mybir.AluOpType.add)
                    if j % 2 == 0:
                        nc.scalar.activation(
                            kt_sb[j][:, sq_i * 512:(sq_i + 1) * 512],
                            pqk[:, 512:1024],
                            mybir.ActivationFunctionType.Identity,
                            bias=bkp[:, j:j + 1], scale=1.0)
                    else:
                        nc.vector.tensor_scalar_add(
                            kt_sb[j][:, sq_i * 512:(sq_i + 1) * 512],
                            pqk[:, 512:1024],
                            bkp[:, j:j + 1])

                # V pass: tokens on psum partitions (x already resident),
                # 8 (sh, jj) blocks continuing the 4-deep tag rotation.
                for b in range(8):
                    sh, jj = b // 4, b % 4
                    pv = ppool.tile([128, 1024], F32, tag=f"t{jj}",
                                    name=f"pv{sh}{jj}")
                    for ct in range(CT):
                        for half in range(2):
                            s0 = sh * 1024 + (2 * jj + half) * 128
                            nc.tensor.matmul(
                                pv[:, half * 512:(half + 1) * 512],
                                lhsT=x_sb[:, ct, s0:s0 + 128],
                                rhs=wv_sb[:, ct, :],
                                start=(ct == 0), stop=(ct == CT - 1))
                    for half in range(2):
                        t_idx = sh * 8 + 2 * jj + half
                        sl = pv[:, half * 512:(half + 1) * 512]
                        # one 8-head add per token tile via 3D APs
                        dst = v4[:, t_idx, :, 0:HD]
                        sl3 = bass.AP(tensor=sl.tensor, offset=sl.offset,
                                      ap=[sl.ap[0], [HD, HPC], [1, HD]])
                        bv3 = bass.AP(tensor=bv_bc.tensor, offset=bv_bc.offset,
                                      ap=[bv_bc.ap[0], [HD, HPC], [1, HD]])
                        nc.vector.tensor_add(dst, sl3, bv3)
                        if apply_mask:
                            nc.vector.tensor_scalar_mul(
                                dst, dst, em_sb[:, t_idx:t_idx + 1])

            # ---------------- Phase 2: attention ----------------
            with ExitStack() as ph2:
                epool = ph2.enter_context(tc.tile_pool(name="ep", bufs=8))
                e01pool = ph2.enter_context(tc.tile_pool(name="e01", bufs=4))
                tpool = ph2.enter_context(tc.tile_pool(name="tp", bufs=3))
                gtpool = ph2.enter_context(tc.tile_pool(name="gt", bufs=3))
                gpool = ph2.enter_context(tc.tile_pool(name="gp", bufs=4))
                opool = ph2.enter_context(tc.tile_pool(name="op", bufs=3))

                for h in range(HPC):
                    qi = h // 2
                    for ih in range(2):
                        blk = h * 2 + ih
                        i0 = ih * 1024
                        ctx_ps = ppool.tile([VW, 1024], F32, tag="t3",
                                            name=f"ctx{blk}")
                        e_tiles = {}
                        emitted = []

                        def _ctx(st):
                            e_t = e_tiles.pop(st)
                            first = not emitted
                            emitted.append(st)
                            last = len(emitted) == ST
                            for q in range(2):
                                nc.tensor.matmul(
                                    ctx_ps[:, q * 512:(q + 1) * 512],
                                    lhsT=v_sb[:, st, h * VW:(h + 1) * VW],
                                    rhs=e_t[:, q * 512:(q + 1) * 512],
                                    start=first, stop=last)

                        for st in range(ST):
                            # per-block tag offset: the first tiles of block
                            # N+1 reuse tags whose exp finished early in block
                            # N — no boundary stall.
                            s_ps = ppool.tile([128, 1024], F32,
                                              tag=f"t{(st + blk) % 3}",
                                              name=f"sc{blk}_{st}")
                            for q in range(2):
                                nc.tensor.matmul(
                                    s_ps[:, q * 512:(q + 1) * 512],
                                    lhsT=kt_sb[qi][:, st * 128:(st + 1) * 128],
                                    rhs=qp_sb[h][:, i0 + q * 512:i0 + (q + 1) * 512],
                                    start=True, stop=True)
                            if st in GP_ST:
                                # exp8q on DVE, the three ^2 passes on GpSimd
                                e_sb = e01pool.tile([128, 1024], BF, tag="e01",
                                                    name=f"eg{blk}_{st}")
                                tmp = gtpool.tile([128, 1024], F32, tag="gtmp",
                                                  name=f"gt{blk}_{st}")
                                nc.vector._custom_dve(
                                    exp8q, out=tmp, in0=s_ps, in1=dcol,
                                    s0=EXP8_A, s1=EXP8_B, imm2=EXP8_G)
                                g1 = gpool.tile([128, 1024], F32, tag="g",
                                                name=f"g1_{blk}_{st}")
                                g2 = gpool.tile([128, 1024], F32, tag="g",
                                                name=f"g2_{blk}_{st}")
                                nc.gpsimd.tensor_mul(g1, tmp, tmp)
                                nc.gpsimd.tensor_mul(g2, g1, g1)
                                nc.gpsimd.tensor_mul(e_sb, g2, g2)
                            elif st in DVE_ST:
                                e_sb = epool.tile([128, 1024], BF, tag="e",
                                                  name=f"e{blk}_{st}")
                                tmp = tpool.tile([128, 1024], F32, tag="tmp",
                                                 name=f"tm{blk}_{st}")
                                nc.vector._custom_dve(
                                    exp8q, out=tmp, in0=s_ps, in1=dcol,
                                    s0=EXP8_A, s1=EXP8_B, imm2=EXP8_G)
                                nc.vector._custom_dve(pow8, out=e_sb, in0=tmp)
                            else:
                                e_sb = epool.tile([128, 1024], BF, tag="e",
                                                  name=f"e{blk}_{st}")
                                nc.scalar.activation(
                                    e_sb, s_ps,
                                    mybir.ActivationFunctionType.Exp,
                                    bias=lnk8, scale=8.0)
                            e_tiles[st] = e_sb
                            if st in CTX_SLOT:
                                _ctx(CTX_SLOT[st])
                        for st in CTX_POST:
                            _ctx(st)
                        o_sb = opool.tile([VW, 1024], F32, tag="o", name=f"o{blk}")
                        nc.vector.tensor_copy(o_sb, ctx_ps)
                        nc.sync.dma_start(
                            out=out[h * VW:(h + 1) * VW, i0:i0 + 1024], in_=o_sb)

    nc.compile()
    return nc


def _get_nc(apply_mask=False):
    key = ("nc", apply_mask)
    if key not in _CACHE:
        _CACHE[key] = _build(apply_mask)
    return _CACHE[key]


def _in_maps(hidden_states, attention_mask, wq, bq, wk, bk, wv, bv):
    ones = np.ones(512, BF16)
    maps = []
    for c in range(8):
        b, g = c // 2, c % 2
        ch0 = g * CH
        maps.append({
            "xt": np.ascontiguousarray(hidden_states[b].T).astype(BF16),
            "wq_t": np.ascontiguousarray(wq[ch0:ch0 + CH, :].T).astype(BF16),
            "wk_t": np.ascontiguousarray(wk[ch0:ch0 + CH, :].T).astype(BF16),
            "wv_t": np.ascontiguousarray(wv[ch0:ch0 + CH, :].T).astype(BF16),
            "bq": np.ascontiguousarray(bq[ch0:ch0 + CH]) / 64.0,
            "bk": np.ascontiguousarray(bk[ch0:ch0 + CH]),
            "bv": np.ascontiguousarray(bv[ch0:ch0 + CH]),
            "mask": np.ascontiguousarray(attention_mask[b, 0, 0, :]),
            "ones": ones,
        })
    return maps


def _gather(results):
    full = np.empty((B, S, H), np.float32)
    for c in range(8):
        b, g = c // 2, c % 2
        o = results[c]["out"].reshape(HPC, VW, S)
        ctx = o[:, :HD, :] / o[:, HD:HD + 1, :]        # normalize by denom row
        # [h, d, s] -> [s, h*d]
        full[b, :, g * CH:(g + 1) * CH] = ctx.reshape(CH, S).T
    return full


def _run(in_maps, trace=False, apply_mask=False):
    from concourse.bass_utils import run_bass_kernel_spmd

    nc = _get_nc(apply_mask)
    return run_bass_kernel_spmd(nc, in_maps, list(range(8)), trace=trace)


def _needs_mask(attention_mask):
    return bool(np.any(np.asarray(attention_mask) != 0.0))


def _run_results(in_maps, apply_mask=False):
    """Run on hardware; on a wedged-device error retry in fresh subprocesses
    (the PJRT client cannot recover an unrecoverable exec unit in-process)."""
    try:
        return _run(in_maps, apply_mask=apply_mask).results
    except Exception:
        pass
    import pickle
    import subprocess
    import tempfile

    last = None
    for _ in range(3):
        try:
            with tempfile.TemporaryDirectory() as td:
                fin = os.path.join(td, "in.pkl")
                fout = os.path.join(td, "out.pkl")
                with open(fin, "wb") as f:
                    pickle.dump((in_maps, apply_mask), f)
                code = (
                    "import pickle, sys\n"
                    f"sys.path.insert(0, {_KERNEL_DIR!r})\n"
                    "import kernel\n"
                    f"maps, am = pickle.load(open({fin!r}, 'rb'))\n"
                    "res = kernel._run(maps, apply_mask=am)\n"
                    f"pickle.dump(res.results, open({fout!r}, 'wb'))\n"
                )
                subprocess.run([sys.executable, "-c", code], check=True,
                               timeout=1800)
                with open(fout, "rb") as f:
                    return pickle.load(f)
        except Exception as e:
            last = e
    raise last


def kernel(hidden_states, attention_mask, wq, bq, wk, bk, wv, bv):
    args = [np.asarray(a, np.float32) for a in
            (hidden_states, attention_mask, wq, bq, wk, bk, wv, bv)]
    am = _needs_mask(args[1])
    return _gather(_run_results(_in_maps(*args), apply_mask=am))


def kernel_profiled(hidden_states, attention_mask, wq, bq, wk, bk, wv, bv):
    """Like kernel() but with NTFF tracing; returns (output, exec_time_ns)."""
    args = [np.asarray(a, np.float32) for a in
            (hidden_states, attention_mask, wq, bq, wk, bk, wv, bv)]
    am = _needs_mask(args[1])
    res = _run(_in_maps(*args), trace=True, apply_mask=am)
    return _gather(res.results), res.exec_time_ns
